# revision 1
# baseline (speedup 1.0000x reference)
"""Distributed Trainium2 kernel for nn_Attention_33002528702591.

Multi-head causal attention with RoPE (B=2, S=2048, D=2048, H=16, HD=128),
run across 8 NeuronCores with a hybrid data/tensor-parallel sharding:
core i handles batch (i // 4) and head group (i % 4) of 4 heads.

Each core computes, for its batch b and its 4 heads:
    QT = (wq_p @ x_b.T)   [512f, S]   (RoPE'd, pre-scaled by 1/sqrt(HD))
    KT = (wk_p @ x_b.T)   [512f, S]   (RoPE'd)
    V  = (x_b @ wv.T)     [S, 512f]
    per head h, q-tile: ST[k,q] = KT_h.T-chunks @ QT_h  (scores, transposed)
                        E = exp(ST) * causal_mask;  colsum = ones.T @ E
                        outT[hd,q] = sum_k V_chunk.T @ E;  outT *= 1/colsum
    partial[dout, t] = woT_slice.T @ attnoutT        [D, S]  (bf16)
The host sums the 4 per-batch partials and transposes back - that is the
"unshard" step for the row-parallel output projection.

No device collectives are needed; all matmuls run in bf16 with fp32 PSUM
accumulation (measured end-to-end rel err vs the fp32 reference ~6e-3).
Activations/weights are cast to bf16 on the host as part of sharding, so
the kernel DMAs matmul operands straight into their SBUF tiles.

Layout trick: everything is kept "feature-on-partition, token-on-free",
with x / weights fed pre-transposed from the host, so the kernel needs no
on-device transposes.  RoPE pairs are made contiguous by permuting wq/wk
ROWS on the host (even hd components first, then odd) - scores are
invariant to a shared permutation of q/k features.
"""

import sys
from contextlib import ExitStack

import numpy as np

if "/opt/trn_rl_repo" not in sys.path:
    sys.path.insert(0, "/opt/trn_rl_repo")

import concourse.bass as bass
import concourse.tile as tile
from concourse import bacc, mybir

F32 = mybir.dt.float32
BF16 = mybir.dt.bfloat16

# problem constants
DIM = 2048
SEQ = 2048
BATCH = 2
N_HEADS = 16
HEAD_DIM = 128
N_CORES = 8
HEADS_PER_CORE = 4  # 2 batches x 4 head-groups = 8 cores

def build_graph(D=DIM, S=SEQ, HC=HEADS_PER_CORE, out_dtype=BF16):
    """One SPMD graph; per-core behavior differs only via input data."""
    HD = HEAD_DIM
    F = HC * HD            # features on this core (512)
    ND = D // 128          # d-chunks (16)
    NT = S // 512          # token tiles (4)
    NF = F // 128          # feature tiles == heads (4)
    DQT = 512              # q tile width

    nc = bacc.Bacc()
    xT = nc.declare_dram_parameter("xT", [D, S], BF16, False)
    wqT = nc.declare_dram_parameter("wqT", [D, F], BF16, False)
    wkT = nc.declare_dram_parameter("wkT", [D, F], BF16, False)
    wvT = nc.declare_dram_parameter("wvT", [D, F], BF16, False)
    woT = nc.declare_dram_parameter("woT", [F, D], BF16, False)
    csq = nc.declare_dram_parameter("csq", [128, S], F32, False)   # [cq;sq] rows
    csk = nc.declare_dram_parameter("csk", [128, S], F32, False)   # [ck;sk] rows
    masks = nc.declare_dram_parameter("masks", [128, 128], BF16, False)
    out = nc.declare_dram_parameter("out", [D, S], out_dtype, True)

    with ExitStack() as ctx:
        tc = ctx.enter_context(tile.TileContext(nc))

        consts = ctx.enter_context(tc.tile_pool(name="consts", bufs=1))
        p_mm = ctx.enter_context(tc.tile_pool(name="p_mm", bufs=6, space="PSUM"))
        p_qk = ctx.enter_context(tc.tile_pool(name="p_qk", bufs=2 * NF))
        p_v = ctx.enter_context(tc.tile_pool(name="p_v", bufs=S // 128))
        p_ao = ctx.enter_context(tc.tile_pool(name="p_ao", bufs=NF))
        p_tmp = ctx.enter_context(tc.tile_pool(name="p_tmp", bufs=6))
        p_w = ctx.enter_context(tc.tile_pool(name="p_w", bufs=3 * ND))
        p_wo = ctx.enter_context(tc.tile_pool(name="p_wo", bufs=NF))
        p_xbf = ctx.enter_context(tc.tile_pool(name="p_xbf", bufs=20))

        # ---- constants (DMAs emitted after the weight/x loads below so the
        # first Q accumulation's data gets queue priority) ----
        csq_sb = consts.tile([128, S], F32, tag="csq")
        csk_sb = consts.tile([128, S], F32, tag="csk")
        masks_sb = consts.tile([128, 128], BF16, tag="masks")
        ones_col = consts.tile([128, 1], BF16, tag="ones_col")
        ones_row = consts.tile([1, 128], BF16, tag="ones_row")
        nc.vector.memset(ones_col[:], 1.0)
        nc.vector.memset(ones_row[:], 1.0)

        # persistent activation tiles
        qt_sb = [p_qk.tile([128, S], BF16, tag="qk", name=f"qt{i}") for i in range(NF)]
        kt_sb = [p_qk.tile([128, S], BF16, tag="qk", name=f"kt{i}") for i in range(NF)]
        v_sb = [p_v.tile([128, F], BF16, tag="v", name=f"v{i}") for i in range(S // 128)]
        ao_sb = [p_ao.tile([128, S], BF16, tag="ao", name=f"ao{i}") for i in range(NF)]

        # weights as bf16; DMA emission order is tuned so the first Q
        # accumulation can start after only a few chunk loads: x(t0) and wq
        # interleave, then wk, then wv.
        wq_bf, wk_bf, wv_bf = [], [], []
        xbf0 = []
        for d in range(ND):
            xb = p_xbf.tile([128, 512], BF16, tag="xbf", name="xb")
            nc.sync.dma_start(out=xb[:], in_=xT[d * 128:(d + 1) * 128, 0:512])
            xbf0.append(xb)
            wbf = p_w.tile([128, F], BF16, tag="w", name=f"wq{d}")
            nc.sync.dma_start(out=wbf[:], in_=wqT[d * 128:(d + 1) * 128, :])
            wq_bf.append(wbf)
        nc.sync.dma_start(out=csq_sb[:], in_=csq[:, :])
        for w_dram, w_list, nm in ((wkT, wk_bf, "k"), (wvT, wv_bf, "v")):
            for d in range(ND):
                wbf = p_w.tile([128, F], BF16, tag="w", name=f"w{nm}{d}")
                nc.sync.dma_start(out=wbf[:], in_=w_dram[d * 128:(d + 1) * 128, :])
                w_list.append(wbf)
            if nm == "k":
                nc.sync.dma_start(out=csk_sb[:], in_=csk[:, :])
        nc.sync.dma_start(out=masks_sb[:], in_=masks[:, :])
        wo_bf = []
        for fc in range(NF):
            wbf = p_wo.tile([128, D], BF16, tag="wo", name=f"wo{fc}")
            nc.sync.dma_start(out=wbf[:], in_=woT[fc * 128:(fc + 1) * 128, :])
            wo_bf.append(wbf)

        # One software pipeline per 512-token tile: QKV(tt) -> attention for
        # every head at q-tile tt (its causal K/V span is fully resident) ->
        # the output-projection columns for tt.  Interleaving the phases keeps
        # ACT(exp) / DVE(RoPE, normalize) / Pool(mask) work available whenever
        # the TensorEngine's own chain stalls.
        p_e = ctx.enter_context(tc.tile_pool(name="p_e", bufs=8))
        p_dr = ctx.enter_context(tc.tile_pool(name="p_dr", bufs=2, space="DRAM"))
        p_acc = ctx.enter_context(tc.tile_pool(name="p_acc", bufs=1, space="PSUM"))
        p_cs = ctx.enter_context(tc.tile_pool(name="p_cs", bufs=1, space="PSUM"))
        p_sm = ctx.enter_context(tc.tile_pool(name="p_sm", bufs=2))
        p_ob = ctx.enter_context(tc.tile_pool(name="p_ob", bufs=4))

        def emit_wo_tile(wt, do):
            wsl = slice(wt * 512, (wt + 1) * 512)
            ps = p_mm.tile([128, 512], F32, tag="mm", name="pso")
            for fc in range(NF):
                nc.tensor.matmul(
                    ps[:],
                    wo_bf[fc][:, do * 128:(do + 1) * 128],
                    ao_sb[fc][:, wsl],
                    start=(fc == 0), stop=(fc == NF - 1),
                )
            ob = p_ob.tile([128, 512], out_dtype, tag="ob", name="ob")
            if do % 2 == 0:
                nc.scalar.copy(ob[:], ps[:])
            else:
                nc.vector.tensor_copy(ob[:], ps[:])
            nc.sync.dma_start(out=out[do * 128:(do + 1) * 128, wsl], in_=ob[:])

        for tt in range(NT):
            tsl = slice(tt * 512, (tt + 1) * 512)
            if tt == 0:
                xbf = xbf0
            else:
                xbf = []
                for d in range(ND):
                    xb = p_xbf.tile([128, 512], BF16, tag="xbf", name="xb")
                    nc.sync.dma_start(out=xb[:], in_=xT[d * 128:(d + 1) * 128, tsl])
                    xbf.append(xb)

            # Q / K projections -> RoPE -> bf16 SBUF
            for w_list, dst, cs_sb in ((wq_bf, qt_sb, csq_sb),
                                       (wk_bf, kt_sb, csk_sb)):
                for ft in range(NF):
                    ps = p_mm.tile([128, 512], F32, tag="mm", name="ps")
                    for d in range(ND):
                        nc.tensor.matmul(
                            ps[:],
                            w_list[d][:, ft * 128:(ft + 1) * 128],
                            xbf[d][:],
                            start=(d == 0),
                            stop=(d == ND - 1),
                        )
                    # RoPE: rows 0:64 = even(ve), 64:128 = odd(vo).  PSUM
                    # operands may pair with SBUF operands at any base; SBUF
                    # pairs must be base-aligned (verifier rule).
                    ve, vo = ps[0:64, :], ps[64:128, :]
                    c, s = cs_sb[0:64, tsl], cs_sb[64:128, tsl]
                    t1 = p_tmp.tile([64, 512], F32, tag="rt", name="t1", bufs=4)
                    t2 = p_tmp.tile([64, 512], F32, tag="rt", name="t2", bufs=4)
                    nc.vector.tensor_mul(t1[:], ve, c)
                    nc.vector.tensor_mul(t2[:], vo, s)
                    # combines on the otherwise-idle GpSimd
                    nc.gpsimd.tensor_sub(dst[ft][0:64, tsl], t1[:], t2[:])
                    t3 = p_tmp.tile([64, 512], F32, tag="rt", name="t3", bufs=4)
                    t4 = p_tmp.tile([64, 512], F32, tag="rt", name="t4", bufs=4)
                    nc.vector.tensor_mul(t3[:], ve, s)
                    nc.vector.tensor_mul(t4[:], vo, c)
                    nc.gpsimd.tensor_add(dst[ft][64:128, tsl], t3[:], t4[:])

            # V projection (layout [t, f])
            for tc4 in range(4):
                tch = tt * 4 + tc4
                ps = p_mm.tile([128, F], F32, tag="mm", name="psv")
                for d in range(ND):
                    nc.tensor.matmul(
                        ps[:],
                        xbf[d][:, tc4 * 128:(tc4 + 1) * 128],
                        wv_bf[d][:],
                        start=(d == 0),
                        stop=(d == ND - 1),
                    )
                nc.scalar.copy(v_sb[tch][:], ps[:])

            # ---- causal attention, q-tile tt for every head, interleaved
            # with the previous tile's output-projection (pure-PE filler
            # for the attention chain's TensorEngine stalls) ----
            qt = tt
            qsl = tsl
            n_kc = 4 * qt + 4  # causal: k chunks 0 .. 4qt+3
            for h in range(HC):
                if tt > 0:
                    for do in range(h * 4, (h + 1) * 4):
                        emit_wo_tile(tt - 1, do)
                outp = p_acc.tile([128, DQT], F32, tag="acc", name="outp")
                cs_ps = p_cs.tile([1, DQT], F32, tag="cs", name="cs_ps")
                for kc in range(n_kc):
                    ksl = slice(kc * 128, (kc + 1) * 128)
                    j = kc - 4 * qt
                    # diagonal chunk j: q-columns [0,128j) are fully
                    # masked (E=0), [128j,128j+128) triangular, rest open
                    qoff = 128 * j if j > 0 else 0
                    st = p_mm.tile([128, DQT], F32, tag="mm", name="st")
                    nc.tensor.matmul(
                        st[:, qoff:], kt_sb[h][:, ksl],
                        qt_sb[h][:, qt * DQT + qoff:(qt + 1) * DQT],
                        start=True, stop=True,
                    )
                    e = p_e.tile([128, DQT], BF16, tag="e", name="e")
                    nc.scalar.activation(
                        e[:, qoff:], st[:, qoff:],
                        mybir.ActivationFunctionType.Exp)
                    if j >= 0:
                        nc.gpsimd.tensor_mul(
                            e[:, qoff:qoff + 128], e[:, qoff:qoff + 128],
                            masks_sb[:])
                    # diagonal chunks contribute nothing to q-columns
                    # [0,qoff): slice PV/colsum to the live region (kc==0 is
                    # always full-width, so the accumulation group is
                    # initialized everywhere)
                    nc.tensor.matmul(
                        outp[:, qoff:], v_sb[kc][:, h * 128:(h + 1) * 128],
                        e[:, qoff:],
                        start=(kc == 0), stop=(kc == n_kc - 1),
                    )
                    nc.tensor.matmul(
                        cs_ps[:, qoff:], ones_col[:], e[:, qoff:],
                        start=(kc == 0), stop=(kc == n_kc - 1),
                    )
                # evict the accumulator to SBUF right away so the single
                # PSUM accumulator slot frees for the next head's PV chain;
                # normalization then runs off the SBUF copy.
                outp_sb = p_sm.tile([128, DQT], F32, tag="osb", name="outp_sb")
                nc.scalar.copy(outp_sb[:], outp[:])
                rcol = p_sm.tile([1, DQT], F32, tag="rcol", name="rcol")
                nc.vector.reciprocal(rcol[:], cs_ps[:])
                rbc = p_sm.tile([128, DQT], F32, tag="rbc", name="rbc")
                if tt == NT - 1 and h == HC - 1:
                    # last head sits on the critical path into the final
                    # output projection: use the lower-latency PE outer
                    # product instead of the DRAM-bounce broadcast
                    rcol_bf = p_sm.tile([1, DQT], BF16, tag="rcolbf",
                                        name="rcol_bf")
                    nc.vector.tensor_copy(rcol_bf[:], rcol[:])
                    rbc_ps = p_mm.tile([128, DQT], F32, tag="mm", name="rbc_ps")
                    nc.tensor.matmul(rbc_ps[:], ones_row[:], rcol_bf[:],
                                     start=True, stop=True)
                    nc.vector.tensor_copy(rbc[:], rbc_ps[:])
                else:
                    # broadcast 1/colsum across partitions via a DRAM bounce
                    # + stride-0-partition DMA read: keeps the broadcast
                    # entirely off the TensorEngine instruction stream
                    rdr = p_dr.tile([1, DQT], F32, tag="rdr", name="rdr")
                    nc.sync.dma_start(out=rdr[:], in_=rcol[:])
                    nc.sync.dma_start(out=rbc[:],
                                      in_=rdr[:].to_broadcast((128, DQT)))
                nc.vector.tensor_mul(ao_sb[h][:, qsl], outp_sb[:], rbc[:])

        # last tile's output projection
        for do in range(ND):
            emit_wo_tile(NT - 1, do)

    nc.finalize()
    return nc


_ROPE_PERM_HEAD = np.concatenate([np.arange(0, HEAD_DIM, 2),
                                  np.arange(1, HEAD_DIM, 2)])


def _rope_perm(n_heads):
    return np.concatenate([h * HEAD_DIM + _ROPE_PERM_HEAD for h in range(n_heads)])


def make_masks():
    """Causal triangle: mask[kl, ql] = 1.0 if ql >= kl else 0 (bf16)."""
    import ml_dtypes
    kl = np.arange(128)[:, None]
    ql = np.arange(128)[None, :]
    return (ql >= kl).astype(np.float32).astype(ml_dtypes.bfloat16)


def make_in_maps(x, freqs_cos, freqs_sin, wq, wk, wv, wo,
                 D=DIM, S=SEQ, HC=HEADS_PER_CORE, n_cores=N_CORES):
    """Shard + relayout the full inputs into per-core input dicts (bf16)."""
    import ml_dtypes
    BF = ml_dtypes.bfloat16
    x = np.asarray(x, np.float32)
    B = x.shape[0]
    F = HC * HEAD_DIM
    n_groups = n_cores // B
    perm = _rope_perm(HC)
    scale = 1.0 / np.sqrt(np.float32(HEAD_DIM))

    cosT = np.ascontiguousarray(np.asarray(freqs_cos, np.float32).T)  # [64, S]
    sinT = np.ascontiguousarray(np.asarray(freqs_sin, np.float32).T)
    csq = np.concatenate([cosT * scale, sinT * scale], 0)  # [128, S]
    csk = np.concatenate([cosT, sinT], 0)
    masks = make_masks()

    xT = [np.ascontiguousarray(x[b].T).astype(BF) for b in range(B)]

    in_maps = []
    for i in range(n_cores):
        b, g = i // n_groups, i % n_groups
        fsl = slice(g * F, (g + 1) * F)
        wq_s = np.asarray(wq, np.float32)[fsl][perm]
        wk_s = np.asarray(wk, np.float32)[fsl][perm]
        wv_s = np.asarray(wv, np.float32)[fsl]
        wo_s = np.asarray(wo, np.float32)[:, fsl]
        in_maps.append({
            "xT": xT[b],
            "wqT": np.ascontiguousarray(wq_s.T).astype(BF),
            "wkT": np.ascontiguousarray(wk_s.T).astype(BF),
            "wvT": np.ascontiguousarray(wv_s.T).astype(BF),
            "woT": np.ascontiguousarray(wo_s.T).astype(BF),
            "csq": csq, "csk": csk, "masks": masks,
        })
    return in_maps


_EXEC_CACHE = None


def _get_executor():
    """Build the graph once and jit-compile the 8-core SPMD executor.

    Mirrors concourse.bass2jax.run_bass_via_pjrt, but cached so repeated
    kernel() calls skip graph construction and lowering.
    """
    global _EXEC_CACHE
    if _EXEC_CACHE is not None:
        return _EXEC_CACHE

    import jax
    from jax.sharding import Mesh, PartitionSpec
    from jax.experimental.shard_map import shard_map
    from concourse import bass2jax, mybir as mb
    from concourse.bass2jax import _bass_exec_p, install_neuronx_cc_hook

    nc = build_graph()
    install_neuronx_cc_hook()
    partition_name = (nc.partition_id_tensor.name
                      if nc.partition_id_tensor else None)
    in_names, out_names, out_avals = [], [], []
    for alloc in nc.m.functions[0].allocations:
        if not isinstance(alloc, mb.MemoryLocationSet):
            continue
        name = alloc.memorylocations[0].name
        if alloc.kind == "ExternalInput":
            if name != partition_name:
                in_names.append(name)
        elif alloc.kind == "ExternalOutput":
            out_names.append(name)
            out_avals.append(jax.core.ShapedArray(
                tuple(alloc.tensor_shape), mb.dt.np(alloc.dtype)))
    n_params = len(in_names)
    n_outs = len(out_avals)
    all_in_names = list(in_names) + list(out_names)
    if partition_name is not None:
        all_in_names.append(partition_name)

    def _body(*args):
        operands = list(args)
        if partition_name is not None:
            operands.append(bass2jax.partition_id_tensor())
        outs = _bass_exec_p.bind(
            *operands,
            out_avals=tuple(out_avals),
            in_names=tuple(all_in_names),
            out_names=tuple(out_names),
            lowering_input_output_aliases=(),
            sim_require_finite=True,
            sim_require_nnan=True,
            nc=nc,
        )
        return tuple(outs)

    devices = jax.devices()[:N_CORES]
    mesh = Mesh(np.asarray(devices), ("core",))
    sharded = jax.jit(
        shard_map(_body, mesh=mesh,
                  in_specs=(PartitionSpec("core"),) * (n_params + n_outs),
                  out_specs=(PartitionSpec("core"),) * n_outs,
                  check_rep=False),
        donate_argnums=tuple(range(n_params, n_params + n_outs)),
        keep_unused=True,
    )
    _EXEC_CACHE = (sharded, in_names, out_names, out_avals, mesh)
    return _EXEC_CACHE


def run_device(in_maps):
    """Run the SPMD kernel; returns per-core output dicts."""
    import jax
    import jax.numpy as jnp
    from jax.sharding import NamedSharding, PartitionSpec

    sharded, in_names, out_names, out_avals, mesh = _get_executor()
    shard = NamedSharding(mesh, PartitionSpec("core"))
    concat_in = [
        np.concatenate([np.asarray(in_maps[c][nm]) for c in range(N_CORES)],
                       axis=0)
        for nm in in_names
    ]
    in_dev = [jax.device_put(a, shard) for a in concat_in]
    zeros = [jnp.zeros((N_CORES * av.shape[0], *av.shape[1:]), av.dtype,
                       device=shard) for av in out_avals]
    out_arrs = sharded(*in_dev, *zeros)
    return [
        {nm: np.asarray(out_arrs[i]).reshape(N_CORES, *out_avals[i].shape)[c]
         for i, nm in enumerate(out_names)}
        for c in range(N_CORES)
    ]


def kernel(x, start_pos, freqs_cos, freqs_sin, mask, wq, wk, wv, wo):
    in_maps = make_in_maps(x, freqs_cos, freqs_sin, wq, wk, wv, wo)
    results = run_device(in_maps)

    B = np.asarray(x).shape[0]
    n_groups = N_CORES // B
    out = np.empty((B, SEQ, DIM), np.float32)
    for b in range(B):
        acc = np.zeros((DIM, SEQ), np.float32)
        for g in range(n_groups):
            acc += np.asarray(results[b * n_groups + g]["out"],
                              dtype=np.float32)
        out[b] = acc.T
    return out



# revision 2
# speedup vs baseline: 1.0081x; 1.0081x over previous
"""Distributed Trainium2 kernel for nn_Attention_33002528702591.

Multi-head causal attention with RoPE (B=2, S=2048, D=2048, H=16, HD=128),
run across 8 NeuronCores with a hybrid data/tensor-parallel sharding:
core i handles batch (i // 4) and head group (i % 4) of 4 heads.

Each core computes, for its batch b and its 4 heads:
    QT = (wq_p @ x_b.T)   [512f, S]   (RoPE'd, pre-scaled by 1/sqrt(HD))
    KT = (wk_p @ x_b.T)   [512f, S]   (RoPE'd)
    V  = (x_b @ wv.T)     [S, 512f]
    per head h, q-tile: ST[k,q] = KT_h.T-chunks @ QT_h  (scores, transposed)
                        E = exp(ST) * causal_mask
                        colsum via E-stationary matmuls with a moving
                        ones[128,1] (engine cost ~1 cycle per chunk)
                        outT[hd,q] = sum_k V_chunk.T @ E;  outT *= 1/colsum
    partial[dout, t] = woT_slice.T @ attnoutT        [D, S]  (bf16)
The host sums the 4 per-batch partials and transposes back - that is the
"unshard" step for the row-parallel output projection.

Schedule: the attention chunk loop is software-pipelined (the scores matmul
for chunk k+1 is emitted before PV of chunk k, so the in-order TensorEngine
queue is never parked behind the ACT exp), and pure-PE filler matmuls are
woven between chunks: the NEXT tile's V projection during tile 0's
attention, the PREVIOUS tile's output projection during tiles 1-3's.
Diagonal chunks split PV into open + masked parts so only the 128-column
masked block waits on the Pool mask-multiply.

No device collectives are needed; all matmuls run in bf16 with fp32 PSUM
accumulation.  Activations/weights are cast to bf16 on the host as part of
sharding.  Layout trick: everything is kept "feature-on-partition,
token-on-free", with x / weights fed pre-transposed from the host; RoPE
pairs are made contiguous by permuting wq/wk ROWS on the host.
"""

import sys
from contextlib import ExitStack

import numpy as np

if "/opt/trn_rl_repo" not in sys.path:
    sys.path.insert(0, "/opt/trn_rl_repo")

import concourse.bass as bass
import concourse.tile as tile
from concourse import bacc, mybir

F32 = mybir.dt.float32
BF16 = mybir.dt.bfloat16

# problem constants
DIM = 2048
SEQ = 2048
BATCH = 2
N_HEADS = 16
HEAD_DIM = 128
N_CORES = 8
HEADS_PER_CORE = 4  # 2 batches x 4 head-groups = 8 cores

def build_graph(D=DIM, S=SEQ, HC=HEADS_PER_CORE, out_dtype=BF16):
    """One SPMD graph; per-core behavior differs only via input data."""
    HD = HEAD_DIM
    F = HC * HD            # features on this core (512)
    ND = D // 128          # d-chunks (16)
    NT = S // 512          # token tiles (4)
    NF = F // 128          # feature tiles == heads (4)
    DQT = 512              # q tile width

    nc = bacc.Bacc()
    xT = nc.declare_dram_parameter("xT", [D, S], BF16, False)
    wqT = nc.declare_dram_parameter("wqT", [D, F], BF16, False)
    wkT = nc.declare_dram_parameter("wkT", [D, F], BF16, False)
    wvT = nc.declare_dram_parameter("wvT", [D, F], BF16, False)
    woT = nc.declare_dram_parameter("woT", [F, D], BF16, False)
    csq = nc.declare_dram_parameter("csq", [128, S], F32, False)   # [cq;sq] rows
    csk = nc.declare_dram_parameter("csk", [128, S], F32, False)   # [ck;sk] rows
    masks = nc.declare_dram_parameter("masks", [128, 128], BF16, False)
    out = nc.declare_dram_parameter("out", [D, S], out_dtype, True)

    with ExitStack() as ctx:
        tc = ctx.enter_context(tile.TileContext(nc))

        consts = ctx.enter_context(tc.tile_pool(name="consts", bufs=1))
        p_mm = ctx.enter_context(tc.tile_pool(name="p_mm", bufs=5, space="PSUM"))
        p_acc = ctx.enter_context(tc.tile_pool(name="p_acc", bufs=2, space="PSUM"))
        p_cs = ctx.enter_context(tc.tile_pool(name="p_cs", bufs=1, space="PSUM"))
        p_qk = ctx.enter_context(tc.tile_pool(name="p_qk", bufs=2 * NF))
        p_v = ctx.enter_context(tc.tile_pool(name="p_v", bufs=S // 128))
        p_ao = ctx.enter_context(tc.tile_pool(name="p_ao", bufs=NF))
        p_tmp = ctx.enter_context(tc.tile_pool(name="p_tmp", bufs=6))
        p_w = ctx.enter_context(tc.tile_pool(name="p_w", bufs=12))
        p_wo = ctx.enter_context(tc.tile_pool(name="p_wo", bufs=NF))
        p_xbf = ctx.enter_context(tc.tile_pool(name="p_xbf", bufs=8))
        p_e = ctx.enter_context(tc.tile_pool(name="p_e", bufs=6))
        p_dr = ctx.enter_context(tc.tile_pool(name="p_dr", bufs=2, space="DRAM"))
        p_sm = ctx.enter_context(tc.tile_pool(name="p_sm", bufs=2))
        p_ob = ctx.enter_context(tc.tile_pool(name="p_ob", bufs=4))

        # ---- constants ----
        csq_sb = consts.tile([128, S], F32, tag="csq", name="csq_sb")
        csk_sb = consts.tile([128, S], F32, tag="csk", name="csk_sb")
        masks_sb = consts.tile([128, 128], BF16, tag="masks", name="masks_sb")
        ones_col = consts.tile([128, 1], BF16, tag="ones_col", name="ones_col")
        ones_row = consts.tile([1, 128], BF16, tag="ones_row", name="ones_row")
        nc.vector.memset(ones_col[:], 1.0)
        nc.vector.memset(ones_row[:], 1.0)

        # persistent activation tiles
        qt_sb = [p_qk.tile([128, S], BF16, tag="qk", name=f"qt{i}") for i in range(NF)]
        kt_sb = [p_qk.tile([128, S], BF16, tag="qk", name=f"kt{i}") for i in range(NF)]
        v_sb = [p_v.tile([128, F], BF16, tag="v", name=f"v{i}") for i in range(S // 128)]
        ao_sb = [p_ao.tile([128, S], BF16, tag="ao", name=f"ao{i}") for i in range(NF)]

        # ---- grouped DMA loads: 4 consecutive 128-row chunks ride one DMA
        # (one HWDGE ring slot instead of four - the ring's ~625ns/DMA is
        # what paces the tile-0 warmup).  The DRAM rows are refolded into a
        # [128, 4*cols] SBUF tile via a (c p) f -> p c f access pattern;
        # chunk d = 4g+c is the [:, c*cols:(c+1)*cols] slice. ----
        def load_x(tt):
            # x rides the ACT engine's DGE ring so it never queues behind the
            # weight stream on SP's ring
            tsl = slice(tt * 512, (tt + 1) * 512)
            xbf = []
            for g in range(4):
                gt = p_xbf.tile([128, 2048], BF16, tag="xbf", name="xbg")
                src = xT[g * 512:(g + 1) * 512, tsl].rearrange(
                    "(c p) t -> p c t", p=128)
                nc.scalar.dma_start(out=gt[:], in_=src)
                xbf.extend(gt[:, c * 512:(c + 1) * 512] for c in range(4))
            return xbf

        def load_w(dram, name, gs):
            chunks = []
            for g in gs:
                gt = p_w.tile([128, 2048], BF16, tag="w", name=f"{name}g{g}")
                src = dram[g * 512:(g + 1) * 512, :].rearrange(
                    "(c p) f -> p c f", p=128)
                nc.sync.dma_start(out=gt[:], in_=src)
                chunks.extend(gt[:, c * F:(c + 1) * F] for c in range(4))
            return chunks

        # tile-0 stream: x/wq groups interleave, csq/csk first slices ride
        # mid-stream, the remaining constant slices follow wv (nothing is
        # starved by then)
        xbf0, wq_bf = [], []
        for g in range(4):
            gt = p_xbf.tile([128, 2048], BF16, tag="xbf", name="xbg")
            src = xT[g * 512:(g + 1) * 512, 0:512].rearrange(
                "(c p) t -> p c t", p=128)
            if g == 0:
                # the very first x/wq bytes gate the whole kernel: load the
                # leading group in 2-chunk halves so the first matmul can
                # start ~1.5us earlier
                nc.scalar.dma_start(out=gt[:, 0:1024], in_=src[:, 0:2])
                nc.scalar.dma_start(out=gt[:, 1024:2048], in_=src[:, 2:4])
            else:
                nc.scalar.dma_start(out=gt[:], in_=src)
            xbf0.extend(gt[:, c * 512:(c + 1) * 512] for c in range(4))
            wq_bf.extend(load_w(wqT, "wq", [g]))
            if g == 2:
                nc.sync.dma_start(out=csq_sb[:, 0:512], in_=csq[:, 0:512])
        wk_bf = []
        for g in range(4):
            wk_bf.extend(load_w(wkT, "wk", [g]))
            if g == 2:
                nc.sync.dma_start(out=csk_sb[:, 0:512], in_=csk[:, 0:512])
        wv_bf = load_w(wvT, "wv", range(4))
        for t in range(1, NT):
            tsl = slice(t * 512, (t + 1) * 512)
            nc.sync.dma_start(out=csq_sb[:, tsl], in_=csq[:, tsl])
            nc.sync.dma_start(out=csk_sb[:, tsl], in_=csk[:, tsl])
        nc.sync.dma_start(out=masks_sb[:], in_=masks[:, :])
        wo_bf = []
        for fc in range(NF):
            wbf = p_wo.tile([128, D], BF16, tag="wo", name=f"wo{fc}")
            nc.sync.dma_start(out=wbf[:], in_=woT[fc * 128:(fc + 1) * 128, :])
            wo_bf.append(wbf)

        def emit_qk(tt, xbf):
            """Q and K projections for tile tt -> RoPE -> bf16 SBUF."""
            tsl = slice(tt * 512, (tt + 1) * 512)
            for w_list, dst, cs_sb in ((wq_bf, qt_sb, csq_sb),
                                       (wk_bf, kt_sb, csk_sb)):
                for ft in range(NF):
                    ps = p_mm.tile([128, 512], F32, tag="mm", name="ps")
                    for d in range(ND):
                        nc.tensor.matmul(
                            ps[:],
                            w_list[d][:, ft * 128:(ft + 1) * 128],
                            xbf[d][:],
                            start=(d == 0),
                            stop=(d == ND - 1),
                        )
                    # RoPE: rows 0:64 = even(ve), 64:128 = odd(vo)
                    ve, vo = ps[0:64, :], ps[64:128, :]
                    c, s = cs_sb[0:64, tsl], cs_sb[64:128, tsl]
                    t1 = p_tmp.tile([64, 512], F32, tag="rt", name="t1", bufs=4)
                    t2 = p_tmp.tile([64, 512], F32, tag="rt", name="t2", bufs=4)
                    nc.vector.tensor_mul(t1[:], ve, c)
                    nc.vector.tensor_mul(t2[:], vo, s)
                    nc.gpsimd.tensor_sub(dst[ft][0:64, tsl], t1[:], t2[:])
                    t3 = p_tmp.tile([64, 512], F32, tag="rt", name="t3", bufs=4)
                    t4 = p_tmp.tile([64, 512], F32, tag="rt", name="t4", bufs=4)
                    nc.vector.tensor_mul(t3[:], ve, s)
                    nc.vector.tensor_mul(t4[:], vo, c)
                    nc.gpsimd.tensor_add(dst[ft][64:128, tsl], t3[:], t4[:])

        def emit_v(tt, xbf):
            """V projection for tile tt (layout [t, f]), proj-phase form."""
            for tc4 in range(4):
                tch = tt * 4 + tc4
                ps = p_mm.tile([128, F], F32, tag="mm", name="psv")
                for d in range(ND):
                    nc.tensor.matmul(
                        ps[:],
                        xbf[d][:, tc4 * 128:(tc4 + 1) * 128],
                        wv_bf[d][:],
                        start=(d == 0),
                        stop=(d == ND - 1),
                    )
                nc.scalar.copy(v_sb[tch][:], ps[:])

        def v_emitters(tt, xbf):
            """Per-matmul emitters for tile tt's V projection (filler form)."""
            for tc4 in range(4):
                tch = tt * 4 + tc4
                ps_box = {}
                for d in range(ND):
                    def emit(tc4=tc4, tch=tch, d=d, ps_box=ps_box):
                        if d == 0:
                            ps_box["ps"] = p_mm.tile([128, F], F32, tag="mm",
                                                     name="psv")
                        nc.tensor.matmul(
                            ps_box["ps"][:],
                            xbf[d][:, tc4 * 128:(tc4 + 1) * 128],
                            wv_bf[d][:],
                            start=(d == 0),
                            stop=(d == ND - 1),
                        )
                        if d == ND - 1:
                            nc.vector.tensor_copy(v_sb[tch][:], ps_box["ps"][:])
                    yield emit

        def wo_emitters(wt):
            """Per-matmul emitters for tile wt's output projection."""
            wsl = slice(wt * 512, (wt + 1) * 512)
            for do in range(ND):
                ps_box = {}
                for fc in range(NF):
                    def emit(do=do, fc=fc, ps_box=ps_box):
                        if fc == 0:
                            ps_box["ps"] = p_mm.tile([128, 512], F32, tag="mm",
                                                     name="pso")
                        nc.tensor.matmul(
                            ps_box["ps"][:],
                            wo_bf[fc][:, do * 128:(do + 1) * 128],
                            ao_sb[fc][:, wsl],
                            start=(fc == 0), stop=(fc == NF - 1),
                        )
                        if fc == NF - 1:
                            ob = p_ob.tile([128, 512], out_dtype, tag="ob",
                                           name="ob")
                            # GPSIMD cannot read PSUM; keep evictions on DVE
                            nc.vector.tensor_copy(ob[:], ps_box["ps"][:])
                            nc.sync.dma_start(
                                out=out[do * 128:(do + 1) * 128, wsl],
                                in_=ob[:])
                    yield emit

        def emit_attention(tt, filler_iter, n_fill, pre_last_normalize=None):
            """Causal attention for q-tile tt, software-pipelined with a
            1-chunk scores lookahead and paced PE fillers."""
            qt = tt
            qsl = slice(tt * 512, (tt + 1) * 512)
            n_kc = 4 * qt + 4
            chunks = [(h, kc) for h in range(HC) for kc in range(n_kc)]
            n_ch = len(chunks)
            st_tiles = {}

            def emit_st(h, kc):
                j = kc - 4 * qt
                qoff = 128 * j if j > 0 else 0
                st = p_mm.tile([128, DQT], F32, tag="mm", name="st")
                nc.tensor.matmul(
                    st[:, qoff:], kt_sb[h][:, kc * 128:(kc + 1) * 128],
                    qt_sb[h][:, qt * DQT + qoff:(qt + 1) * DQT],
                    start=True, stop=True,
                )
                st_tiles[(h, kc)] = st

            outp_t, cs_t = {}, {}
            pv_started, cs_started = {}, {}
            taken = 0
            emit_st(*chunks[0])
            emit_st(*chunks[1])
            for i, (h, kc) in enumerate(chunks):
                j = kc - 4 * qt
                qoff = 128 * j if j > 0 else 0
                last_head = (tt == NT - 1 and h == HC - 1)
                if kc == 0:
                    outp_t[h] = p_acc.tile([128, DQT], F32, tag="acc",
                                           name="outp")
                    pv_started[h] = False
                    cs_started[h] = False
                    if last_head:
                        cs_t[h] = p_cs.tile([1, DQT], F32, tag="cs",
                                            name="cs_ps")
                    else:
                        cs_t[h] = p_cs.tile([128, 4], F32, tag="cs",
                                            name="cs4")
                outp = outp_t[h]

                def pv_mm(lo, hi, stop, h=h, kc=kc, outp=outp):
                    st_flag = not pv_started[h]
                    pv_started[h] = True
                    nc.tensor.matmul(
                        outp[:, lo:hi], v_sb[kc][:, h * 128:(h + 1) * 128],
                        e[:, lo:hi], start=st_flag, stop=stop,
                    )

                def cs_mm(c, stop, h=h):
                    st_flag = not cs_started[h]
                    cs_started[h] = True
                    nc.tensor.matmul(
                        cs_t[h][:, c:c + 1], e[:, c * 128:(c + 1) * 128],
                        ones_col[:], start=st_flag, stop=stop,
                    )

                st = st_tiles.pop((h, kc))
                e = p_e.tile([128, DQT], BF16, tag="e", name="e")
                nc.scalar.activation(
                    e[:, qoff:], st[:, qoff:],
                    mybir.ActivationFunctionType.Exp)
                if j >= 0:
                    nc.gpsimd.tensor_mul(
                        e[:, qoff:qoff + 128], e[:, qoff:qoff + 128],
                        masks_sb[:])
                # scores lookahead (depth 2): later chunks' STs reach the PE
                # before this chunk's PV, so exp latency hides behind them
                if i + 2 < n_ch:
                    emit_st(*chunks[i + 2])
                # paced pure-PE fillers (shifted one chunk late so the
                # attention start doesn't contend with the projection
                # phase's RoPE-pending PSUM tiles)
                want = (n_fill * i) // n_ch
                while taken < want:
                    next(filler_iter)()
                    taken += 1
                # PV + colsum.  For diagonal chunks, the open region
                # [qoff+128:) does not depend on the mask multiply - emit it
                # first so only the masked 128-block waits on the Pool hop.
                if j >= 0:
                    if qoff + 128 < DQT:
                        pv_mm(qoff + 128, DQT, False)
                    if last_head:
                        pv_mm(qoff, qoff + 128, kc == n_kc - 1)
                        nc.tensor.matmul(
                            cs_t[h][:, qoff:], ones_col[:], e[:, qoff:],
                            start=(kc == 0), stop=(kc == n_kc - 1),
                        )
                    else:
                        for c in range(j + 1, 4):
                            cs_mm(c, False)
                        pv_mm(qoff, qoff + 128, kc == n_kc - 1)
                        cs_mm(j, j == 3)
                else:
                    pv_mm(0, DQT, False)
                    if last_head:
                        nc.tensor.matmul(
                            cs_t[h][:], ones_col[:], e[:],
                            start=(kc == 0), stop=False,
                        )
                    else:
                        for c in range(4):
                            cs_mm(c, False)
                if kc == n_kc - 1:
                    # normalize: reciprocal first (it unblocks the broadcast
                    # chain), accumulator eviction after, all off the PE
                    rbc = p_sm.tile([128, DQT], F32, tag="rbc", name="rbc")
                    if not last_head:
                        # transposing DRAM bounce: rdr[c, p] = rc4[p, c] =
                        # recip(q = 128c + p), so the broadcast read is a
                        # contiguous stride-0-partition AP
                        rc4 = p_sm.tile([128, 4], F32, tag="rc4", name="rc4")
                        nc.vector.reciprocal(rc4[:], cs_t[h][:])
                        rdr = p_dr.tile([4, 128], F32, tag="rdr", name="rdr")
                        nc.sync.dma_start(out=rdr[:, :].transpose([1, 0]),
                                          in_=rc4[:])
                        nc.sync.dma_start(
                            out=rbc[:],
                            in_=rdr[:, :].flatten().unsqueeze(0)
                            .to_broadcast((128, DQT)))
                        outp_sb = p_sm.tile([128, DQT], F32, tag="osb",
                                            name="outp_sb")
                        nc.vector.tensor_copy(outp_sb[:], outp[:])
                    else:
                        # last head sits on the critical path into the final
                        # output projection: PE outer-product broadcast
                        # instead of the slower DRAM bounce, with the first
                        # output-projection chains as PE filler while the
                        # reciprocal runs on the DVE
                        rcol = p_sm.tile([1, DQT], F32, tag="rcol",
                                         name="rcol", bufs=1)
                        nc.vector.reciprocal(rcol[:], cs_t[h][:])
                        rcol_bf = p_sm.tile([1, DQT], BF16, tag="rcolbf",
                                            name="rcol_bf", bufs=1)
                        nc.vector.tensor_copy(rcol_bf[:], rcol[:])
                        if pre_last_normalize is not None:
                            pre_last_normalize()
                        rbc_ps = p_mm.tile([128, DQT], F32, tag="mm",
                                           name="rbc_ps")
                        nc.tensor.matmul(rbc_ps[:], ones_row[:], rcol_bf[:],
                                         start=True, stop=True)
                        outp_sb = p_sm.tile([128, DQT], F32, tag="osb",
                                            name="outp_sb")
                        # ACT is exp-idle by now; evict there so the DVE can
                        # turn rbc around immediately
                        nc.scalar.copy(outp_sb[:], outp[:])
                        nc.vector.tensor_copy(rbc[:], rbc_ps[:])
                    nc.vector.tensor_mul(ao_sb[h][:, qsl], outp_sb[:], rbc[:])
            while taken < n_fill:
                next(filler_iter)()
                taken += 1

        # ================= main schedule =================
        emit_qk(0, xbf0)
        emit_v(0, xbf0)
        xbf1 = load_x(1)
        emit_attention(0, v_emitters(1, xbf1), 64)

        emit_qk(1, xbf1)
        emit_attention(1, wo_emitters(0), 64)

        xbf2 = load_x(2)
        emit_qk(2, xbf2)
        emit_v(2, xbf2)
        emit_attention(2, wo_emitters(1), 64)

        xbf3 = load_x(3)
        emit_qk(3, xbf3)
        emit_v(3, xbf3)

        # last tile's output projection is split: each do-chain's first three
        # head contributions (ready before the last head finishes) can run as
        # PE filler during the last head's normalize; only the fc=3 matmuls
        # wait on the final ao
        wsl3 = slice(3 * 512, 4 * 512)
        wo3_ps = {}

        def wo3_open(do):
            ps = p_mm.tile([128, 512], F32, tag="mm", name="pso")
            wo3_ps[do] = ps
            for fc in range(3):
                nc.tensor.matmul(ps[:], wo_bf[fc][:, do * 128:(do + 1) * 128],
                                 ao_sb[fc][:, wsl3],
                                 start=(fc == 0), stop=False)

        def wo3_close(do):
            # evictions alternate DVE/ACT (ACT is exp-idle by now); all DMAs
            # stay on SP's ring so ACT's sequencer never blocks a copy behind
            # a DMA-issue slot.  The final do is split in half across both
            # engines to shorten the kernel's tail.
            ps = wo3_ps.pop(do)
            nc.tensor.matmul(ps[:], wo_bf[3][:, do * 128:(do + 1) * 128],
                             ao_sb[3][:, wsl3], start=False, stop=True)
            ob = p_ob.tile([128, 512], out_dtype, tag="ob", name="ob")
            osl = slice(do * 128, (do + 1) * 128)
            if do == ND - 1:
                nc.vector.tensor_copy(ob[:, 0:256], ps[:, 0:256])
                nc.scalar.copy(ob[:, 256:512], ps[:, 256:512])
                nc.sync.dma_start(out=out[osl, 3 * 512:3 * 512 + 256],
                                  in_=ob[:, 0:256])
                nc.sync.dma_start(out=out[osl, 3 * 512 + 256:4 * 512],
                                  in_=ob[:, 256:512])
            else:
                if do % 2 == 0:
                    nc.vector.tensor_copy(ob[:], ps[:])
                else:
                    nc.scalar.copy(ob[:], ps[:])
                nc.sync.dma_start(out=out[osl, wsl3], in_=ob[:])

        def pre_tail():
            wo3_open(0)
            wo3_open(1)

        emit_attention(3, wo_emitters(2), 64, pre_last_normalize=pre_tail)

        wo3_open(2)
        wo3_open(3)
        for do in range(4):
            wo3_close(do)
        for do in range(4, ND):
            wo3_open(do)
            wo3_close(do)

    nc.finalize()
    return nc


_ROPE_PERM_HEAD = np.concatenate([np.arange(0, HEAD_DIM, 2),
                                  np.arange(1, HEAD_DIM, 2)])


def _rope_perm(n_heads):
    return np.concatenate([h * HEAD_DIM + _ROPE_PERM_HEAD for h in range(n_heads)])


def make_masks():
    """Causal triangle: mask[kl, ql] = 1.0 if ql >= kl else 0 (bf16)."""
    import ml_dtypes
    kl = np.arange(128)[:, None]
    ql = np.arange(128)[None, :]
    return (ql >= kl).astype(np.float32).astype(ml_dtypes.bfloat16)


def make_in_maps(x, freqs_cos, freqs_sin, wq, wk, wv, wo,
                 D=DIM, S=SEQ, HC=HEADS_PER_CORE, n_cores=N_CORES):
    """Shard + relayout the full inputs into per-core input dicts (bf16)."""
    import ml_dtypes
    BF = ml_dtypes.bfloat16
    x = np.asarray(x, np.float32)
    B = x.shape[0]
    F = HC * HEAD_DIM
    n_groups = n_cores // B
    perm = _rope_perm(HC)
    scale = 1.0 / np.sqrt(np.float32(HEAD_DIM))

    cosT = np.ascontiguousarray(np.asarray(freqs_cos, np.float32).T)  # [64, S]
    sinT = np.ascontiguousarray(np.asarray(freqs_sin, np.float32).T)
    csq = np.concatenate([cosT * scale, sinT * scale], 0)  # [128, S]
    csk = np.concatenate([cosT, sinT], 0)
    masks = make_masks()

    xT = [np.ascontiguousarray(x[b].T).astype(BF) for b in range(B)]

    in_maps = []
    for i in range(n_cores):
        b, g = i // n_groups, i % n_groups
        fsl = slice(g * F, (g + 1) * F)
        wq_s = np.asarray(wq, np.float32)[fsl][perm]
        wk_s = np.asarray(wk, np.float32)[fsl][perm]
        wv_s = np.asarray(wv, np.float32)[fsl]
        wo_s = np.asarray(wo, np.float32)[:, fsl]
        in_maps.append({
            "xT": xT[b],
            "wqT": np.ascontiguousarray(wq_s.T).astype(BF),
            "wkT": np.ascontiguousarray(wk_s.T).astype(BF),
            "wvT": np.ascontiguousarray(wv_s.T).astype(BF),
            "woT": np.ascontiguousarray(wo_s.T).astype(BF),
            "csq": csq, "csk": csk, "masks": masks,
        })
    return in_maps


_EXEC_CACHE = None


def _get_executor():
    """Build the graph once and jit-compile the 8-core SPMD executor.

    Mirrors concourse.bass2jax.run_bass_via_pjrt, but cached so repeated
    kernel() calls skip graph construction and lowering.
    """
    global _EXEC_CACHE
    if _EXEC_CACHE is not None:
        return _EXEC_CACHE

    import jax
    from jax.sharding import Mesh, PartitionSpec
    from jax.experimental.shard_map import shard_map
    from concourse import bass2jax, mybir as mb
    from concourse.bass2jax import _bass_exec_p, install_neuronx_cc_hook

    nc = build_graph()
    install_neuronx_cc_hook()
    partition_name = (nc.partition_id_tensor.name
                      if nc.partition_id_tensor else None)
    in_names, out_names, out_avals = [], [], []
    for alloc in nc.m.functions[0].allocations:
        if not isinstance(alloc, mb.MemoryLocationSet):
            continue
        name = alloc.memorylocations[0].name
        if alloc.kind == "ExternalInput":
            if name != partition_name:
                in_names.append(name)
        elif alloc.kind == "ExternalOutput":
            out_names.append(name)
            out_avals.append(jax.core.ShapedArray(
                tuple(alloc.tensor_shape), mb.dt.np(alloc.dtype)))
    n_params = len(in_names)
    n_outs = len(out_avals)
    all_in_names = list(in_names) + list(out_names)
    if partition_name is not None:
        all_in_names.append(partition_name)

    def _body(*args):
        operands = list(args)
        if partition_name is not None:
            operands.append(bass2jax.partition_id_tensor())
        outs = _bass_exec_p.bind(
            *operands,
            out_avals=tuple(out_avals),
            in_names=tuple(all_in_names),
            out_names=tuple(out_names),
            lowering_input_output_aliases=(),
            sim_require_finite=True,
            sim_require_nnan=True,
            nc=nc,
        )
        return tuple(outs)

    devices = jax.devices()[:N_CORES]
    mesh = Mesh(np.asarray(devices), ("core",))
    sharded = jax.jit(
        shard_map(_body, mesh=mesh,
                  in_specs=(PartitionSpec("core"),) * (n_params + n_outs),
                  out_specs=(PartitionSpec("core"),) * n_outs,
                  check_rep=False),
        donate_argnums=tuple(range(n_params, n_params + n_outs)),
        keep_unused=True,
    )
    _EXEC_CACHE = (sharded, in_names, out_names, out_avals, mesh)
    return _EXEC_CACHE


def run_device(in_maps):
    """Run the SPMD kernel; returns per-core output dicts."""
    import jax
    import jax.numpy as jnp
    from jax.sharding import NamedSharding, PartitionSpec

    sharded, in_names, out_names, out_avals, mesh = _get_executor()
    shard = NamedSharding(mesh, PartitionSpec("core"))
    concat_in = [
        np.concatenate([np.asarray(in_maps[c][nm]) for c in range(N_CORES)],
                       axis=0)
        for nm in in_names
    ]
    in_dev = [jax.device_put(a, shard) for a in concat_in]
    zeros = [jnp.zeros((N_CORES * av.shape[0], *av.shape[1:]), av.dtype,
                       device=shard) for av in out_avals]
    out_arrs = sharded(*in_dev, *zeros)
    return [
        {nm: np.asarray(out_arrs[i]).reshape(N_CORES, *out_avals[i].shape)[c]
         for i, nm in enumerate(out_names)}
        for c in range(N_CORES)
    ]


def kernel(x, start_pos, freqs_cos, freqs_sin, mask, wq, wk, wv, wo):
    in_maps = make_in_maps(x, freqs_cos, freqs_sin, wq, wk, wv, wo)
    results = run_device(in_maps)

    B = np.asarray(x).shape[0]
    n_groups = N_CORES // B
    out = np.empty((B, SEQ, DIM), np.float32)
    for b in range(B):
        acc = np.zeros((DIM, SEQ), np.float32)
        for g in range(n_groups):
            acc += np.asarray(results[b * n_groups + g]["out"],
                              dtype=np.float32)
        out[b] = acc.T
    return out
